# revision 18
# baseline (speedup 1.0000x reference)
"""GCNConv layer on 8 Trainium2 NeuronCores (Bass/Tile) — v2.

out = relu( D^-1/2 (A+I) D^-1/2 (x W) + b ) + x
    = relu( (dinv_d * (sum_{e->d} dinv_s x_s + dinv_d x_d)) @ W + b ) + x
(W applied after aggregation by linearity).

Each core owns N/8 destination nodes; sources split into 4 chunks of N/4
rows (dma_gather int16 index limit). Per chunk: destinations ordered by
in-degree from that chunk (ELL prefixes), k-th incoming edge of every dst
forms a prefix. Fused dma_gathers of <=4096 slots (passes split at 128
boundaries), round-robin over the 4 SWDGE queues (desc-gen parallelism
across Q7 core pairs is the bottleneck: ~9.4ns/desc per queue pair).
Gathered rows are scaled by dinv_src (DVE, host blob) and accumulated in
one full-shard SBUF accumulator per chunk (rank order, unscaled by dst).
After each chunk: accumulator written densely to HBM (HWDGE), then
per-group merge gathers (natural dst order <- chunk rank order) issued
immediately so they interleave with the next chunk's edge gathers.
Merged group sums stay in SBUF; final per group: scale by dinv_d, add
dinv_d^2 x_d self-loop, PE transpose, matmul W, ACT bias+relu, PE
transpose back, add residual x, store natural-order output.

Edges are padded with weight-0 fake slots so all 8 cores run the same
static SPMD program with per-core data only.
"""

import sys
import types

sys.path.insert(0, "/opt/trn_rl_repo")

import numpy as np

DIM = 64
N_CORES = 8
N_CHUNKS = 4
N_QUEUES = 4
P = 128
GB = 8          # dst blocks per merge/final group
FUSE_CAP = 4096  # max slots per fused gather


def _install_ntff_hook():
    if "antenv.axon_hooks" in sys.modules:
        return
    try:
        sys.path.insert(0, "/root/.axon_site")
        from trn_agent_boot.trn_boot import _ntff_profile_via_ctypes

        hook = _ntff_profile_via_ctypes("/opt/axon/libaxon_pjrt.so")
    except Exception:
        hook = None
    mod = types.ModuleType("antenv.axon_hooks")
    mod.get_axon_ntff_profile_hook = lambda: hook
    mod.set_axon_ntff_profile_hook = lambda h: None
    sys.modules["antenv.axon_hooks"] = mod


class Plan:
    def __init__(self, n_nodes, n_cores, n_chunks):
        assert n_nodes % n_cores == 0
        assert n_nodes % n_chunks == 0
        self.N = n_nodes
        self.n_cores = n_cores
        self.n_chunks = n_chunks
        self.SHARD = n_nodes // n_cores
        self.CH = n_nodes // n_chunks
        assert self.CH <= 32767, "chunk must fit int16 index"
        self.SHB = -(-self.SHARD // P)
        self.SLOTS = self.SHB * P
        self.n_groups = -(-self.SHB // GB)
        self.group_sizes = [
            min(GB, self.SHB - g * GB) * P for g in range(self.n_groups)
        ]
        self.MG_COLS = sum(s // 16 for s in self.group_sizes) * n_chunks
        self.pass_sizes = None   # [chunk] -> list of padded pass sizes
        self.fuse = None         # [chunk] -> list of (segments, slots)
        self.g16_off = None
        self.g128_off = None
        self.GCOLS = 0
        self.WCOLS = 0

    def mg_off(self, g, c):
        o = 0
        for gg in range(g):
            o += (self.group_sizes[gg] // 16) * self.n_chunks
        return o + (self.group_sizes[g] // 16) * c


def _rep16(vals_i16, n):
    a = np.asarray(vals_i16, dtype=np.int16).reshape(n // 16, 16).T
    return np.tile(a, (8, 1))


def preprocess(x, edge_index, W, b):
    x = np.ascontiguousarray(np.asarray(x, dtype=np.float32))
    N = x.shape[0]
    plan = Plan(N, N_CORES, N_CHUNKS)
    src = np.asarray(edge_index[0], dtype=np.int64)
    dst = np.asarray(edge_index[1], dtype=np.int64)
    deg = np.bincount(dst, minlength=N).astype(np.float64) + 1.0
    dinv = (1.0 / np.sqrt(deg)).astype(np.float32)

    SHARD, CH = plan.SHARD, plan.CH

    core_of = dst // SHARD
    per_core = []
    for i in range(N_CORES):
        m = core_of == i
        s_i = src[m]
        d_i = dst[m] - i * SHARD
        c_i = s_i // CH
        chunks = []
        for c in range(N_CHUNKS):
            mm = c_i == c
            s = s_i[mm]
            d = d_i[mm]
            deg_ch = np.bincount(d, minlength=SHARD)
            order = np.argsort(-deg_ch, kind="stable")
            rank = np.empty(SHARD, dtype=np.int64)
            rank[order] = np.arange(SHARD)
            perm = np.argsort(rank[d], kind="stable")
            s_sorted = s[perm]
            counts = deg_ch[order]
            cum = np.concatenate([[0], np.cumsum(counts)])
            K = int(counts[0]) if len(s) else 0
            passes = []
            for k in range(K):
                L = int(np.searchsorted(-counts, -k, side="left"))
                passes.append(s_sorted[cum[:L] + k])
            chunks.append({"passes": passes, "rank": rank})
        per_core.append(chunks)

    # shared (SPMD) padded pass sizes per chunk
    pass_sizes = []
    for c in range(N_CHUNKS):
        K = max(len(per_core[i][c]["passes"]) for i in range(N_CORES))
        sizes = []
        for k in range(K):
            L = max(
                len(per_core[i][c]["passes"][k])
                if k < len(per_core[i][c]["passes"])
                else 0
                for i in range(N_CORES)
            )
            sizes.append(-(-L // P) * P)
        pass_sizes.append(sizes)
    plan.pass_sizes = pass_sizes

    # fused gather groups (passes split at 128 boundaries, cap FUSE_CAP)
    fuse = []
    for c in range(N_CHUNKS):
        groups = []
        segs, tot = [], 0
        for k, n in enumerate(pass_sizes[c]):
            off = 0
            while off < n:
                take = min(FUSE_CAP - tot, n - off)
                segs.append((k, off, take))
                tot += take
                off += take
                if tot == FUSE_CAP:
                    groups.append((segs, tot))
                    segs, tot = [], 0
        if tot:
            groups.append((segs, tot))
        fuse.append(groups)
    plan.fuse = fuse

    g16_off, g128_off = [], []
    o16 = o128 = 0
    for c in range(N_CHUNKS):
        offs16, offs128 = [], []
        for n in pass_sizes[c]:
            offs16.append(o16)
            offs128.append(o128)
            o16 += n // 16
            o128 += n // P
        g16_off.append(offs16)
        g128_off.append(offs128)
    plan.g16_off, plan.g128_off = g16_off, g128_off
    plan.GCOLS = max(o16, 16)
    plan.WCOLS = max(o128, 1)

    W = np.ascontiguousarray(np.asarray(W, dtype=np.float32))
    b = np.ascontiguousarray(np.asarray(b, dtype=np.float32).reshape(DIM, 1))
    in_maps = []
    for i in range(N_CORES):
        gidx = np.zeros((P, plan.GCOLS), dtype=np.int16)
        gwgt = np.zeros((P, plan.WCOLS), dtype=np.float32)
        for c in range(N_CHUNKS):
            pdata = per_core[i][c]
            for k, n in enumerate(pass_sizes[c]):
                s_pass = (
                    pdata["passes"][k]
                    if k < len(pdata["passes"])
                    else np.empty(0, np.int64)
                )
                L = len(s_pass)
                iv = np.zeros(n, dtype=np.int16)
                wv = np.zeros(n, dtype=np.float32)
                iv[:L] = (s_pass - c * CH).astype(np.int16)
                wv[:L] = dinv[s_pass]
                gidx[:, plan.g16_off[c][k] : plan.g16_off[c][k] + n // 16] = (
                    _rep16(iv, n)
                )
                gwgt[:, plan.g128_off[c][k] : plan.g128_off[c][k] + n // P] = (
                    wv.reshape(n // P, P).T
                )
        # merge indices: natural dst order -> chunk rank
        mgidx = np.zeros((P, plan.MG_COLS), dtype=np.int16)
        for g in range(plan.n_groups):
            gsz = plan.group_sizes[g]
            d = np.arange(g * GB * P, g * GB * P + gsz)
            dc = np.clip(d, 0, SHARD - 1)
            for c in range(N_CHUNKS):
                rank = per_core[i][c]["rank"]
                iv = rank[dc].astype(np.int16)
                iv[d >= SHARD] = 0
                o = plan.mg_off(g, c)
                mgidx[:, o : o + gsz // 16] = _rep16(iv, gsz)
        # natural-order dst scale blobs (position d -> partition d%128, col d//128)
        dvd = np.zeros((plan.SLOTS,), dtype=np.float32)
        dvd[:SHARD] = dinv[i * SHARD : (i + 1) * SHARD]
        dinvd = np.ascontiguousarray(dvd.reshape(plan.SHB, P).T)
        dvs = np.zeros((plan.SLOTS,), dtype=np.float32)
        dvs[:SHARD] = dinv[i * SHARD : (i + 1) * SHARD] ** 2
        dinvsq = np.ascontiguousarray(dvs.reshape(plan.SHB, P).T)
        xsh = np.zeros((plan.SLOTS, DIM), dtype=np.float32)
        xsh[:SHARD] = x[i * SHARD : (i + 1) * SHARD]
        in_maps.append(
            {
                "x": x,
                "xsh": xsh,
                "w": W,
                "bias": b,
                "dinvd": dinvd,
                "dinvsq": dinvsq,
                "gidx": gidx,
                "gwgt": gwgt,
                "mgidx": mgidx,
            }
        )
    return plan, in_maps


_QPATCHED = [False]


def _patch_queue_aware_dma_lanes():
    """Partition the 8 DMASW completion-sem lanes so SWDGE queue q owns
    lanes {2q, 2q+1} (Tile's round-robin ignores queue_num; queues sharing a
    lane can complete out of order and release waiters early)."""
    if _QPATCHED[0]:
        return
    _QPATCHED[0] = True
    from concourse import tile_sem_assignment as tsa
    from concourse import bass_isa, mybir

    orig = tsa.TileClockTick._assign_tick

    def qaware(self, inst):
        if (
            isinstance(inst, tsa.DMAInst)
            and inst.engine == mybir.EngineType.Pool
            and not isinstance(inst, bass_isa.UserSyncedRemoteDMADescs)
        ):
            qn = getattr(inst, "queue_num", 0) or 0
            tog = getattr(self, "_q_toggle", None)
            if tog is None:
                tog = self._q_toggle = {}
            t = tog.get(qn, 0)
            tog[qn] = t ^ 1
            self.next_sw_dma_idx = 2 * qn + t
        return orig(self, inst)

    tsa.TileClockTick._assign_tick = qaware


def build_program(plan):
    from concourse import bacc, mybir
    import concourse.tile as tile
    from concourse.masks import make_identity
    from concourse.tile import add_dep_helper

    _patch_queue_aware_dma_lanes()

    N = plan.N
    SHARD, CH = plan.SHARD, plan.CH
    SHB = plan.SHB
    f32 = mybir.dt.float32
    i16 = mybir.dt.int16
    mult = mybir.AluOpType.mult
    add = mybir.AluOpType.add

    nc = bacc.Bacc("TRN2", target_bir_lowering=False, num_swdge_queues=N_QUEUES)
    x_d = nc.dram_tensor("x", [N, DIM], f32, kind="ExternalInput")
    xsh_d = nc.dram_tensor("xsh", [plan.SLOTS, DIM], f32, kind="ExternalInput")
    w_d = nc.dram_tensor("w", [DIM, DIM], f32, kind="ExternalInput")
    b_d = nc.dram_tensor("bias", [DIM, 1], f32, kind="ExternalInput")
    dinvd_d = nc.dram_tensor("dinvd", [P, SHB], f32, kind="ExternalInput")
    dinvsq_d = nc.dram_tensor("dinvsq", [P, SHB], f32, kind="ExternalInput")
    gidx_d = nc.dram_tensor("gidx", [P, plan.GCOLS], i16, kind="ExternalInput")
    gwgt_d = nc.dram_tensor("gwgt", [P, plan.WCOLS], f32, kind="ExternalInput")
    mgidx_d = nc.dram_tensor("mgidx", [P, plan.MG_COLS], i16, kind="ExternalInput")
    accd = [
        nc.dram_tensor(f"accd{c}", [plan.SLOTS, DIM], f32)
        for c in range(N_CHUNKS)
    ]
    out_d = nc.dram_tensor("out", [SHARD, DIM], f32, kind="ExternalOutput")

    qctr = [0]

    def next_q():
        q = qctr[0] % N_QUEUES
        qctr[0] += 1
        return q

    with tile.TileContext(nc) as tc:
        with (
            tc.tile_pool(name="const", bufs=1) as constp,
            tc.tile_pool(name="io", bufs=2) as iop,
            tc.tile_pool(name="gbuf", bufs=8) as gbufp,
            tc.tile_pool(name="mbuf", bufs=6) as mbufp,
            tc.tile_pool(name="mgacc", bufs=1) as mgaccp,
            tc.tile_pool(name="accp", bufs=2) as accp,
            tc.tile_pool(name="xg", bufs=2) as xgp,
            tc.tile_pool(name="fin", bufs=2) as finp,
            tc.tile_pool(name="psum", bufs=2, space="PSUM") as psump,
            tc.tile_pool(name="psum1", bufs=1, space="PSUM") as psum1p,
        ):
            # chunk 0 gather metadata first so desc-gen can start ASAP
            c0cols16 = plan.g16_off[1][0] if N_CHUNKS > 1 else plan.GCOLS
            gidx_t0 = iop.tile([P, c0cols16], i16, tag="gidx")
            nc.sync.dma_start(out=gidx_t0[:], in_=gidx_d[:, :c0cols16])

            ident = constp.tile([P, P], f32)
            make_identity(nc, ident[:])
            w_t = constp.tile([DIM, DIM], f32)
            nc.sync.dma_start(out=w_t[:], in_=w_d[:])
            b_t = constp.tile([DIM, 1], f32)
            nc.sync.dma_start(out=b_t[:], in_=b_d[:])
            dinvd_t = constp.tile([P, SHB], f32)
            nc.sync.dma_start(out=dinvd_t[:], in_=dinvd_d[:])
            dinvsq_t = constp.tile([P, SHB], f32)
            nc.sync.dma_start(out=dinvsq_t[:], in_=dinvsq_d[:])
            mgidx_t = constp.tile([P, plan.MG_COLS], i16)
            nc.sync.dma_start(out=mgidx_t[:], in_=mgidx_d[:])

            mg_tiles = [None] * plan.n_groups

            for c in range(N_CHUNKS):
                o16c = plan.g16_off[c][0]
                o128c = plan.g128_off[c][0]
                if c == 0:
                    gidx_t = gidx_t0
                else:
                    cols16 = (
                        plan.g16_off[c + 1][0] if c + 1 < N_CHUNKS else plan.GCOLS
                    ) - o16c
                    gidx_t = iop.tile([P, cols16], i16, tag="gidx")
                    nc.sync.dma_start(
                        out=gidx_t[:], in_=gidx_d[:, o16c : o16c + cols16]
                    )
                wcols = (
                    plan.g128_off[c + 1][0] if c + 1 < N_CHUNKS else plan.WCOLS
                ) - o128c
                gwgt_t = iop.tile([P, wcols], f32, tag="gwgt")
                nc.sync.dma_start(
                    out=gwgt_t[:], in_=gwgt_d[:, o128c : o128c + wcols]
                )

                acc_t = accp.tile([P, SHB * DIM], f32, tag="acc")
                nc.vector.memset(acc_t[:], 0.0)

                for gi, (segs, slots) in enumerate(plan.fuse[c]):
                    nblk = slots // P
                    buf = gbufp.tile([P, (FUSE_CAP // P) * DIM], f32, tag="gb")
                    k0, off0, _ = segs[0]
                    s16 = plan.g16_off[c][k0] - o16c + off0 // 16
                    s128 = plan.g128_off[c][k0] - o128c + off0 // P
                    nc.gpsimd.dma_gather(
                        out_ap=buf[:, : nblk * DIM].rearrange(
                            "p (j d) -> p j d", d=DIM
                        ),
                        in_ap=x_d[c * CH : (c + 1) * CH, :],
                        idxs_ap=gidx_t[:, s16 : s16 + slots // 16],
                        num_idxs=slots,
                        num_idxs_reg=slots,
                        elem_size=DIM,
                        single_packet=False,
                        queue_num=next_q(),
                    )
                    nc.vector.tensor_tensor(
                        out=buf[:, : nblk * DIM].rearrange("p (j d) -> p j d", d=DIM),
                        in0=buf[:, : nblk * DIM].rearrange("p (j d) -> p j d", d=DIM),
                        in1=gwgt_t[:, s128 : s128 + nblk].to_broadcast(
                            [P, nblk, DIM]
                        ),
                        op=mult,
                    )
                    boff = 0
                    for k, off, ln in segs:
                        nb = ln // P
                        p0 = off // P
                        nc.vector.tensor_tensor(
                            out=acc_t[:, p0 * DIM : (p0 + nb) * DIM],
                            in0=acc_t[:, p0 * DIM : (p0 + nb) * DIM],
                            in1=buf[:, boff * DIM : (boff + nb) * DIM],
                            op=add,
                        )
                        boff += nb
                winst = nc.sync.dma_start(
                    out=accd[c][:, :].rearrange("(j p) d -> p j d", p=P),
                    in_=acc_t[:].rearrange("p (j d) -> p j d", d=DIM),
                )

                # merge gathers for this chunk (interleave with next chunk)
                for g in range(plan.n_groups):
                    gsz = plan.group_sizes[g]
                    blks = gsz // P
                    mb = mbufp.tile([P, GB * DIM], f32, tag="mb")
                    o = plan.mg_off(g, c)
                    ginst = nc.gpsimd.dma_gather(
                        out_ap=mb[:, : blks * DIM].rearrange(
                            "p (j d) -> p j d", d=DIM
                        ),
                        in_ap=accd[c][:, :],
                        idxs_ap=mgidx_t[:, o : o + gsz // 16],
                        num_idxs=gsz,
                        num_idxs_reg=gsz,
                        elem_size=DIM,
                        single_packet=False,
                        queue_num=next_q(),
                    )
                    add_dep_helper(ginst.ins, winst.ins, reason="accd before merge")
                    if c == 0:
                        mg = mgaccp.tile([P, GB * DIM], f32, tag=f"mg{g}")
                        mg_tiles[g] = mg
                        nc.vector.tensor_copy(
                            out=mg[:, : blks * DIM], in_=mb[:, : blks * DIM]
                        )
                    else:
                        mg = mg_tiles[g]
                        nc.vector.tensor_tensor(
                            out=mg[:, : blks * DIM],
                            in0=mg[:, : blks * DIM],
                            in1=mb[:, : blks * DIM],
                            op=add,
                        )

            # final epilogue per group (natural dst order)
            for g in range(plan.n_groups):
                gsz = plan.group_sizes[g]
                blks = gsz // P
                mg = mg_tiles[g]
                xg = xgp.tile([P, GB * DIM], f32, tag="xg")
                nc.sync.dma_start(
                    out=xg[:, : blks * DIM].rearrange("p (bb d) -> p bb d", d=DIM),
                    in_=xsh_d[g * GB * P : g * GB * P + gsz, :].rearrange(
                        "(bb p) d -> p bb d", p=P
                    ),
                )
                ag = finp.tile([P, GB * DIM], f32, tag="ag")
                # ag = dinvd * mg + dinvsq * xg
                nc.vector.tensor_tensor(
                    out=ag[:, : blks * DIM].rearrange("p (bb d) -> p bb d", d=DIM),
                    in0=mg[:, : blks * DIM].rearrange("p (bb d) -> p bb d", d=DIM),
                    in1=dinvd_t[:, g * GB : g * GB + blks].to_broadcast(
                        [P, blks, DIM]
                    ),
                    op=mult,
                )
                sl = finp.tile([P, GB * DIM], f32, tag="sl")
                nc.vector.tensor_tensor(
                    out=sl[:, : blks * DIM].rearrange("p (bb d) -> p bb d", d=DIM),
                    in0=xg[:, : blks * DIM].rearrange("p (bb d) -> p bb d", d=DIM),
                    in1=dinvsq_t[:, g * GB : g * GB + blks].to_broadcast(
                        [P, blks, DIM]
                    ),
                    op=mult,
                )
                nc.vector.tensor_tensor(
                    out=ag[:, : blks * DIM],
                    in0=ag[:, : blks * DIM],
                    in1=sl[:, : blks * DIM],
                    op=add,
                )
                pt = psump.tile([DIM, GB * P], f32, tag="pt")
                for bb in range(blks):
                    nc.tensor.transpose(
                        out=pt[:, bb * P : (bb + 1) * P],
                        in_=ag[:, bb * DIM : (bb + 1) * DIM],
                        identity=ident[:],
                    )
                at = finp.tile([DIM, GB * P], f32, tag="at")
                nc.scalar.activation(
                    out=at[:, : blks * P],
                    in_=pt[:, : blks * P],
                    func=mybir.ActivationFunctionType.Copy,
                )
                pz = psum1p.tile([DIM, GB * P], f32, tag="pz")
                for mo in range(0, blks * P, 512):
                    mw = min(512, blks * P - mo)
                    nc.tensor.matmul(
                        out=pz[:, mo : mo + mw],
                        lhsT=w_t[:],
                        rhs=at[:, mo : mo + mw],
                        start=True,
                        stop=True,
                    )
                zr = finp.tile([DIM, GB * P], f32, tag="zr")
                nc.scalar.activation(
                    out=zr[:, : blks * P],
                    in_=pz[:, : blks * P],
                    func=mybir.ActivationFunctionType.Relu,
                    bias=b_t[:],
                )
                po = psump.tile([P, GB * DIM], f32, tag="po")
                for bb in range(blks):
                    nc.tensor.transpose(
                        out=po[:, bb * DIM : (bb + 1) * DIM],
                        in_=zr[:, bb * P : (bb + 1) * P],
                        identity=ident[:DIM, :DIM],
                    )
                ot = finp.tile([P, GB * DIM], f32, tag="ot")
                nc.vector.tensor_tensor(
                    out=ot[:, : blks * DIM],
                    in0=po[:, : blks * DIM],
                    in1=xg[:, : blks * DIM],
                    op=add,
                )
                row0 = g * GB * P
                rows = min(SHARD - row0, blks * P)
                fb2 = rows // P
                if fb2:
                    nc.sync.dma_start(
                        out=out_d[row0 : row0 + fb2 * P, :].rearrange(
                            "(bb p) d -> p bb d", p=P
                        ),
                        in_=ot[:, : fb2 * DIM].rearrange("p (bb d) -> p bb d", d=DIM),
                    )
                rem2 = rows - fb2 * P
                if rem2:
                    nc.sync.dma_start(
                        out=out_d[row0 + fb2 * P : row0 + rows, :],
                        in_=ot[:rem2, fb2 * DIM : (fb2 + 1) * DIM],
                    )

    nc.compile()
    return nc


def run(plan, nc, in_maps, trace=False, tmpdir=None):
    _install_ntff_hook()
    from concourse.bass_utils import run_bass_kernel_spmd

    res = run_bass_kernel_spmd(
        nc,
        in_maps,
        core_ids=list(range(plan.n_cores)),
        trace=trace,
        tmpdir=tmpdir,
    )
    outs = [res.results[i]["out"] for i in range(plan.n_cores)]
    return np.concatenate(outs, axis=0), res


_CACHE = {}


def kernel(x, edge_index, W, b):
    plan, in_maps = preprocess(x, edge_index, W, b)
    sig = tuple(tuple(s) for s in plan.pass_sizes)
    ent = _CACHE.get("prog")
    if ent is None or ent[0] != sig:
        nc = build_program(plan)
        _CACHE["prog"] = (sig, nc)
    nc = _CACHE["prog"][1]
    out, _ = run(plan, nc, in_maps)
    return out


# revision 20
# speedup vs baseline: 1.0012x; 1.0012x over previous
"""GCNConv layer on 8 Trainium2 NeuronCores (Bass/Tile) — v2.

out = relu( D^-1/2 (A+I) D^-1/2 (x W) + b ) + x
    = relu( (dinv_d * (sum_{e->d} dinv_s x_s + dinv_d x_d)) @ W + b ) + x
(W applied after aggregation by linearity).

Each core owns N/8 destination nodes; sources split into 4 chunks of N/4
rows (dma_gather int16 index limit). Per chunk: destinations ordered by
in-degree from that chunk (ELL prefixes), k-th incoming edge of every dst
forms a prefix. Fused dma_gathers of <=4096 slots (passes split at 128
boundaries), round-robin over the 4 SWDGE queues (desc-gen parallelism
across Q7 core pairs is the bottleneck: ~9.4ns/desc per queue pair).
Gathered rows are scaled by dinv_src (DVE, host blob) and accumulated in
one full-shard SBUF accumulator per chunk (rank order, unscaled by dst).
After each chunk: accumulator written densely to HBM (HWDGE), then
per-group merge gathers (natural dst order <- chunk rank order) issued
immediately so they interleave with the next chunk's edge gathers.
Merged group sums stay in SBUF; final per group: scale by dinv_d, add
dinv_d^2 x_d self-loop, PE transpose, matmul W, ACT bias+relu, PE
transpose back, add residual x, store natural-order output.

Edges are padded with weight-0 fake slots so all 8 cores run the same
static SPMD program with per-core data only.
"""

import sys
import types

sys.path.insert(0, "/opt/trn_rl_repo")

import numpy as np

DIM = 64
N_CORES = 8
N_CHUNKS = 4
N_QUEUES = 4
P = 128
GB = 8          # dst blocks per merge/final group
FUSE_CAP = 4096  # max slots per fused gather


def _install_ntff_hook():
    if "antenv.axon_hooks" in sys.modules:
        return
    try:
        sys.path.insert(0, "/root/.axon_site")
        from trn_agent_boot.trn_boot import _ntff_profile_via_ctypes

        hook = _ntff_profile_via_ctypes("/opt/axon/libaxon_pjrt.so")
    except Exception:
        hook = None
    mod = types.ModuleType("antenv.axon_hooks")
    mod.get_axon_ntff_profile_hook = lambda: hook
    mod.set_axon_ntff_profile_hook = lambda h: None
    sys.modules["antenv.axon_hooks"] = mod


class Plan:
    def __init__(self, n_nodes, n_cores, n_chunks):
        assert n_nodes % n_cores == 0
        assert n_nodes % n_chunks == 0
        self.N = n_nodes
        self.n_cores = n_cores
        self.n_chunks = n_chunks
        self.SHARD = n_nodes // n_cores
        self.CH = n_nodes // n_chunks
        assert self.CH <= 32767, "chunk must fit int16 index"
        self.SHB = -(-self.SHARD // P)
        self.SLOTS = self.SHB * P
        self.n_groups = -(-self.SHB // GB)
        self.group_sizes = [
            min(GB, self.SHB - g * GB) * P for g in range(self.n_groups)
        ]
        self.MG_COLS = sum(s // 16 for s in self.group_sizes) * n_chunks
        self.pass_sizes = None   # [chunk] -> list of padded pass sizes
        self.fuse = None         # [chunk] -> list of (segments, slots)
        self.g16_off = None
        self.g128_off = None
        self.GCOLS = 0
        self.WCOLS = 0

    def mg_off(self, g, c):
        o = 0
        for gg in range(g):
            o += (self.group_sizes[gg] // 16) * self.n_chunks
        return o + (self.group_sizes[g] // 16) * c


def _rep16(vals_i16, n):
    a = np.asarray(vals_i16, dtype=np.int16).reshape(n // 16, 16).T
    return np.tile(a, (8, 1))


def preprocess(x, edge_index, W, b):
    x = np.ascontiguousarray(np.asarray(x, dtype=np.float32))
    N = x.shape[0]
    plan = Plan(N, N_CORES, N_CHUNKS)
    src = np.asarray(edge_index[0], dtype=np.int64)
    dst = np.asarray(edge_index[1], dtype=np.int64)
    deg = np.bincount(dst, minlength=N).astype(np.float64) + 1.0
    dinv = (1.0 / np.sqrt(deg)).astype(np.float32)

    SHARD, CH = plan.SHARD, plan.CH

    core_of = dst // SHARD
    per_core = []
    for i in range(N_CORES):
        m = core_of == i
        s_i = src[m]
        d_i = dst[m] - i * SHARD
        c_i = s_i // CH
        chunks = []
        for c in range(N_CHUNKS):
            mm = c_i == c
            s = s_i[mm]
            d = d_i[mm]
            deg_ch = np.bincount(d, minlength=SHARD)
            order = np.argsort(-deg_ch, kind="stable")
            rank = np.empty(SHARD, dtype=np.int64)
            rank[order] = np.arange(SHARD)
            perm = np.argsort(rank[d], kind="stable")
            s_sorted = s[perm]
            counts = deg_ch[order]
            cum = np.concatenate([[0], np.cumsum(counts)])
            K = int(counts[0]) if len(s) else 0
            passes = []
            for k in range(K):
                L = int(np.searchsorted(-counts, -k, side="left"))
                passes.append(s_sorted[cum[:L] + k])
            chunks.append({"passes": passes, "rank": rank})
        per_core.append(chunks)

    # shared (SPMD) padded pass sizes per chunk
    pass_sizes = []
    for c in range(N_CHUNKS):
        K = max(len(per_core[i][c]["passes"]) for i in range(N_CORES))
        sizes = []
        for k in range(K):
            L = max(
                len(per_core[i][c]["passes"][k])
                if k < len(per_core[i][c]["passes"])
                else 0
                for i in range(N_CORES)
            )
            sizes.append(-(-L // P) * P)
        pass_sizes.append(sizes)
    plan.pass_sizes = pass_sizes

    # fused gather groups (passes split at 128 boundaries, cap FUSE_CAP)
    fuse = []
    for c in range(N_CHUNKS):
        groups = []
        segs, tot = [], 0
        for k, n in enumerate(pass_sizes[c]):
            off = 0
            while off < n:
                take = min(FUSE_CAP - tot, n - off)
                segs.append((k, off, take))
                tot += take
                off += take
                if tot == FUSE_CAP:
                    groups.append((segs, tot))
                    segs, tot = [], 0
        if tot:
            groups.append((segs, tot))
        fuse.append(groups)
    plan.fuse = fuse

    g16_off, g128_off = [], []
    o16 = o128 = 0
    for c in range(N_CHUNKS):
        offs16, offs128 = [], []
        for n in pass_sizes[c]:
            offs16.append(o16)
            offs128.append(o128)
            o16 += n // 16
            o128 += n // P
        g16_off.append(offs16)
        g128_off.append(offs128)
    plan.g16_off, plan.g128_off = g16_off, g128_off
    plan.GCOLS = max(o16, 16)
    plan.WCOLS = max(o128, 1)

    W = np.ascontiguousarray(np.asarray(W, dtype=np.float32))
    b = np.ascontiguousarray(np.asarray(b, dtype=np.float32).reshape(DIM, 1))
    in_maps = []
    for i in range(N_CORES):
        gidx = np.zeros((P, plan.GCOLS), dtype=np.int16)
        gwgt = np.zeros((P, plan.WCOLS), dtype=np.float32)
        for c in range(N_CHUNKS):
            pdata = per_core[i][c]
            for k, n in enumerate(pass_sizes[c]):
                s_pass = (
                    pdata["passes"][k]
                    if k < len(pdata["passes"])
                    else np.empty(0, np.int64)
                )
                L = len(s_pass)
                iv = np.zeros(n, dtype=np.int16)
                wv = np.zeros(n, dtype=np.float32)
                iv[:L] = (s_pass - c * CH).astype(np.int16)
                wv[:L] = dinv[s_pass]
                gidx[:, plan.g16_off[c][k] : plan.g16_off[c][k] + n // 16] = (
                    _rep16(iv, n)
                )
                gwgt[:, plan.g128_off[c][k] : plan.g128_off[c][k] + n // P] = (
                    wv.reshape(n // P, P).T
                )
        # merge indices: natural dst order -> chunk rank
        mgidx = np.zeros((P, plan.MG_COLS), dtype=np.int16)
        for g in range(plan.n_groups):
            gsz = plan.group_sizes[g]
            d = np.arange(g * GB * P, g * GB * P + gsz)
            dc = np.clip(d, 0, SHARD - 1)
            for c in range(N_CHUNKS):
                rank = per_core[i][c]["rank"]
                iv = rank[dc].astype(np.int16)
                iv[d >= SHARD] = 0
                o = plan.mg_off(g, c)
                mgidx[:, o : o + gsz // 16] = _rep16(iv, gsz)
        # natural-order dst scale blobs (position d -> partition d%128, col d//128)
        dvd = np.zeros((plan.SLOTS,), dtype=np.float32)
        dvd[:SHARD] = dinv[i * SHARD : (i + 1) * SHARD]
        dinvd = np.ascontiguousarray(dvd.reshape(plan.SHB, P).T)
        dvs = np.zeros((plan.SLOTS,), dtype=np.float32)
        dvs[:SHARD] = dinv[i * SHARD : (i + 1) * SHARD] ** 2
        dinvsq = np.ascontiguousarray(dvs.reshape(plan.SHB, P).T)
        xsh = np.zeros((plan.SLOTS, DIM), dtype=np.float32)
        xsh[:SHARD] = x[i * SHARD : (i + 1) * SHARD]
        in_maps.append(
            {
                "x": x,
                "xsh": xsh,
                "w": W,
                "bias": b,
                "dinvd": dinvd,
                "dinvsq": dinvsq,
                "gidx": gidx,
                "gwgt": gwgt,
                "mgidx": mgidx,
            }
        )
    return plan, in_maps


_QPATCHED = [False]


def _patch_queue_aware_dma_lanes():
    """Partition the 8 DMASW completion-sem lanes so SWDGE queue q owns
    lanes {2q, 2q+1} (Tile's round-robin ignores queue_num; queues sharing a
    lane can complete out of order and release waiters early)."""
    if _QPATCHED[0]:
        return
    _QPATCHED[0] = True
    from concourse import tile_sem_assignment as tsa
    from concourse import bass_isa, mybir

    orig = tsa.TileClockTick._assign_tick

    def qaware(self, inst):
        if (
            isinstance(inst, tsa.DMAInst)
            and inst.engine == mybir.EngineType.Pool
            and not isinstance(inst, bass_isa.UserSyncedRemoteDMADescs)
        ):
            qn = getattr(inst, "queue_num", 0) or 0
            tog = getattr(self, "_q_toggle", None)
            if tog is None:
                tog = self._q_toggle = {}
            t = tog.get(qn, 0)
            tog[qn] = t ^ 1
            self.next_sw_dma_idx = 2 * qn + t
        return orig(self, inst)

    tsa.TileClockTick._assign_tick = qaware


def build_program(plan):
    from concourse import bacc, mybir
    import concourse.tile as tile
    from concourse.masks import make_identity
    from concourse.tile import add_dep_helper

    _patch_queue_aware_dma_lanes()

    N = plan.N
    SHARD, CH = plan.SHARD, plan.CH
    SHB = plan.SHB
    f32 = mybir.dt.float32
    i16 = mybir.dt.int16
    mult = mybir.AluOpType.mult
    add = mybir.AluOpType.add

    nc = bacc.Bacc("TRN2", target_bir_lowering=False, num_swdge_queues=N_QUEUES)
    x_d = nc.dram_tensor("x", [N, DIM], f32, kind="ExternalInput")
    xsh_d = nc.dram_tensor("xsh", [plan.SLOTS, DIM], f32, kind="ExternalInput")
    w_d = nc.dram_tensor("w", [DIM, DIM], f32, kind="ExternalInput")
    b_d = nc.dram_tensor("bias", [DIM, 1], f32, kind="ExternalInput")
    dinvd_d = nc.dram_tensor("dinvd", [P, SHB], f32, kind="ExternalInput")
    dinvsq_d = nc.dram_tensor("dinvsq", [P, SHB], f32, kind="ExternalInput")
    gidx_d = nc.dram_tensor("gidx", [P, plan.GCOLS], i16, kind="ExternalInput")
    gwgt_d = nc.dram_tensor("gwgt", [P, plan.WCOLS], f32, kind="ExternalInput")
    mgidx_d = nc.dram_tensor("mgidx", [P, plan.MG_COLS], i16, kind="ExternalInput")
    accd = [
        nc.dram_tensor(f"accd{c}", [plan.SLOTS, DIM], f32)
        for c in range(N_CHUNKS)
    ]
    out_d = nc.dram_tensor("out", [SHARD, DIM], f32, kind="ExternalOutput")

    qctr = [0]

    def next_q():
        q = qctr[0] % N_QUEUES
        qctr[0] += 1
        return q

    with tile.TileContext(nc) as tc:
        with (
            tc.tile_pool(name="const", bufs=1) as constp,
            tc.tile_pool(name="io", bufs=2) as iop,
            tc.tile_pool(name="gbuf", bufs=8) as gbufp,
            tc.tile_pool(name="mbuf", bufs=7) as mbufp,
            tc.tile_pool(name="mgacc", bufs=1) as mgaccp,
            tc.tile_pool(name="accp", bufs=2) as accp,
            tc.tile_pool(name="xg", bufs=2) as xgp,
            tc.tile_pool(name="fin", bufs=2) as finp,
            tc.tile_pool(name="psum", bufs=2, space="PSUM") as psump,
            tc.tile_pool(name="psum1", bufs=1, space="PSUM") as psum1p,
        ):
            # chunk 0 gather metadata first so desc-gen can start ASAP
            c0cols16 = plan.g16_off[1][0] if N_CHUNKS > 1 else plan.GCOLS
            gidx_t0 = iop.tile([P, c0cols16], i16, tag="gidx")
            nc.sync.dma_start(out=gidx_t0[:], in_=gidx_d[:, :c0cols16])

            ident = constp.tile([P, P], f32)
            make_identity(nc, ident[:])
            w_t = constp.tile([DIM, DIM], f32)
            nc.sync.dma_start(out=w_t[:], in_=w_d[:])
            b_t = constp.tile([DIM, 1], f32)
            nc.sync.dma_start(out=b_t[:], in_=b_d[:])
            dinvd_t = constp.tile([P, SHB], f32)
            nc.sync.dma_start(out=dinvd_t[:], in_=dinvd_d[:])
            dinvsq_t = constp.tile([P, SHB], f32)
            nc.sync.dma_start(out=dinvsq_t[:], in_=dinvsq_d[:])
            mgidx_t = constp.tile([P, plan.MG_COLS], i16)
            nc.sync.dma_start(out=mgidx_t[:], in_=mgidx_d[:])

            mg_tiles = [None] * plan.n_groups
            rcap = nc.gpsimd.to_reg(FUSE_CAP)

            for c in range(N_CHUNKS):
                o16c = plan.g16_off[c][0]
                o128c = plan.g128_off[c][0]
                if c == 0:
                    gidx_t = gidx_t0
                else:
                    cols16 = (
                        plan.g16_off[c + 1][0] if c + 1 < N_CHUNKS else plan.GCOLS
                    ) - o16c
                    gidx_t = iop.tile([P, cols16], i16, tag="gidx")
                    nc.sync.dma_start(
                        out=gidx_t[:], in_=gidx_d[:, o16c : o16c + cols16]
                    )
                wcols = (
                    plan.g128_off[c + 1][0] if c + 1 < N_CHUNKS else plan.WCOLS
                ) - o128c
                gwgt_t = iop.tile([P, wcols], f32, tag="gwgt")
                nc.sync.dma_start(
                    out=gwgt_t[:], in_=gwgt_d[:, o128c : o128c + wcols]
                )

                acc_t = accp.tile([P, SHB * DIM], f32, tag="acc")
                nc.vector.memset(acc_t[:], 0.0)

                for gi, (segs, slots) in enumerate(plan.fuse[c]):
                    nblk = slots // P
                    buf = gbufp.tile([P, (FUSE_CAP // P) * DIM], f32, tag="gb")
                    k0, off0, _ = segs[0]
                    s16 = plan.g16_off[c][k0] - o16c + off0 // 16
                    s128 = plan.g128_off[c][k0] - o128c + off0 // P
                    nc.gpsimd.dma_gather(
                        out_ap=buf[:, : nblk * DIM].rearrange(
                            "p (j d) -> p j d", d=DIM
                        ),
                        in_ap=x_d[c * CH : (c + 1) * CH, :],
                        idxs_ap=gidx_t[:, s16 : s16 + slots // 16],
                        num_idxs=slots,
                        num_idxs_reg=rcap if slots == FUSE_CAP else slots,
                        elem_size=DIM,
                        single_packet=False,
                        queue_num=next_q(),
                    )
                    nc.vector.tensor_tensor(
                        out=buf[:, : nblk * DIM].rearrange("p (j d) -> p j d", d=DIM),
                        in0=buf[:, : nblk * DIM].rearrange("p (j d) -> p j d", d=DIM),
                        in1=gwgt_t[:, s128 : s128 + nblk].to_broadcast(
                            [P, nblk, DIM]
                        ),
                        op=mult,
                    )
                    boff = 0
                    for k, off, ln in segs:
                        nb = ln // P
                        p0 = off // P
                        nc.vector.tensor_tensor(
                            out=acc_t[:, p0 * DIM : (p0 + nb) * DIM],
                            in0=acc_t[:, p0 * DIM : (p0 + nb) * DIM],
                            in1=buf[:, boff * DIM : (boff + nb) * DIM],
                            op=add,
                        )
                        boff += nb
                winst = nc.sync.dma_start(
                    out=accd[c][:, :].rearrange("(j p) d -> p j d", p=P),
                    in_=acc_t[:].rearrange("p (j d) -> p j d", d=DIM),
                )

                # merge gathers for this chunk (interleave with next chunk)
                for g in range(plan.n_groups):
                    gsz = plan.group_sizes[g]
                    blks = gsz // P
                    mb = mbufp.tile([P, GB * DIM], f32, tag="mb")
                    o = plan.mg_off(g, c)
                    ginst = nc.gpsimd.dma_gather(
                        out_ap=mb[:, : blks * DIM].rearrange(
                            "p (j d) -> p j d", d=DIM
                        ),
                        in_ap=accd[c][:, :],
                        idxs_ap=mgidx_t[:, o : o + gsz // 16],
                        num_idxs=gsz,
                        num_idxs_reg=gsz,
                        elem_size=DIM,
                        single_packet=False,
                        queue_num=next_q(),
                    )
                    add_dep_helper(ginst.ins, winst.ins, reason="accd before merge")
                    if c == 0:
                        mg = mgaccp.tile([P, GB * DIM], f32, tag=f"mg{g}")
                        mg_tiles[g] = mg
                        nc.vector.tensor_copy(
                            out=mg[:, : blks * DIM], in_=mb[:, : blks * DIM]
                        )
                    else:
                        mg = mg_tiles[g]
                        nc.vector.tensor_tensor(
                            out=mg[:, : blks * DIM],
                            in0=mg[:, : blks * DIM],
                            in1=mb[:, : blks * DIM],
                            op=add,
                        )

            # final epilogue per group (natural dst order)
            for g in range(plan.n_groups):
                gsz = plan.group_sizes[g]
                blks = gsz // P
                mg = mg_tiles[g]
                xg = xgp.tile([P, GB * DIM], f32, tag="xg")
                nc.sync.dma_start(
                    out=xg[:, : blks * DIM].rearrange("p (bb d) -> p bb d", d=DIM),
                    in_=xsh_d[g * GB * P : g * GB * P + gsz, :].rearrange(
                        "(bb p) d -> p bb d", p=P
                    ),
                )
                ag = finp.tile([P, GB * DIM], f32, tag="ag")
                # ag = dinvd * mg + dinvsq * xg
                nc.vector.tensor_tensor(
                    out=ag[:, : blks * DIM].rearrange("p (bb d) -> p bb d", d=DIM),
                    in0=mg[:, : blks * DIM].rearrange("p (bb d) -> p bb d", d=DIM),
                    in1=dinvd_t[:, g * GB : g * GB + blks].to_broadcast(
                        [P, blks, DIM]
                    ),
                    op=mult,
                )
                sl = finp.tile([P, GB * DIM], f32, tag="sl")
                nc.vector.tensor_tensor(
                    out=sl[:, : blks * DIM].rearrange("p (bb d) -> p bb d", d=DIM),
                    in0=xg[:, : blks * DIM].rearrange("p (bb d) -> p bb d", d=DIM),
                    in1=dinvsq_t[:, g * GB : g * GB + blks].to_broadcast(
                        [P, blks, DIM]
                    ),
                    op=mult,
                )
                nc.vector.tensor_tensor(
                    out=ag[:, : blks * DIM],
                    in0=ag[:, : blks * DIM],
                    in1=sl[:, : blks * DIM],
                    op=add,
                )
                pt = psum1p.tile([DIM, GB * P], f32, tag="pt")
                for bb in range(blks):
                    nc.tensor.transpose(
                        out=pt[:, bb * P : (bb + 1) * P],
                        in_=ag[:, bb * DIM : (bb + 1) * DIM],
                        identity=ident[:],
                    )
                at = finp.tile([DIM, GB * P], f32, tag="at", bufs=1)
                nc.scalar.activation(
                    out=at[:, : blks * P],
                    in_=pt[:, : blks * P],
                    func=mybir.ActivationFunctionType.Copy,
                )
                pz = psum1p.tile([DIM, GB * P], f32, tag="pz")
                for mo in range(0, blks * P, 512):
                    mw = min(512, blks * P - mo)
                    nc.tensor.matmul(
                        out=pz[:, mo : mo + mw],
                        lhsT=w_t[:],
                        rhs=at[:, mo : mo + mw],
                        start=True,
                        stop=True,
                    )
                zr = finp.tile([DIM, GB * P], f32, tag="zr")
                nc.scalar.activation(
                    out=zr[:, : blks * P],
                    in_=pz[:, : blks * P],
                    func=mybir.ActivationFunctionType.Relu,
                    bias=b_t[:],
                )
                po = psump.tile([P, GB * DIM], f32, tag="po")
                for bb in range(blks):
                    nc.tensor.transpose(
                        out=po[:, bb * DIM : (bb + 1) * DIM],
                        in_=zr[:, bb * P : (bb + 1) * P],
                        identity=ident[:DIM, :DIM],
                    )
                ot = finp.tile([P, GB * DIM], f32, tag="ot")
                nc.vector.tensor_tensor(
                    out=ot[:, : blks * DIM],
                    in0=po[:, : blks * DIM],
                    in1=xg[:, : blks * DIM],
                    op=add,
                )
                row0 = g * GB * P
                rows = min(SHARD - row0, blks * P)
                fb2 = rows // P
                if fb2:
                    nc.sync.dma_start(
                        out=out_d[row0 : row0 + fb2 * P, :].rearrange(
                            "(bb p) d -> p bb d", p=P
                        ),
                        in_=ot[:, : fb2 * DIM].rearrange("p (bb d) -> p bb d", d=DIM),
                    )
                rem2 = rows - fb2 * P
                if rem2:
                    nc.sync.dma_start(
                        out=out_d[row0 + fb2 * P : row0 + rows, :],
                        in_=ot[:rem2, fb2 * DIM : (fb2 + 1) * DIM],
                    )

    nc.compile()
    return nc


def run(plan, nc, in_maps, trace=False, tmpdir=None):
    _install_ntff_hook()
    from concourse.bass_utils import run_bass_kernel_spmd

    res = run_bass_kernel_spmd(
        nc,
        in_maps,
        core_ids=list(range(plan.n_cores)),
        trace=trace,
        tmpdir=tmpdir,
    )
    outs = [res.results[i]["out"] for i in range(plan.n_cores)]
    return np.concatenate(outs, axis=0), res


_CACHE = {}


def kernel(x, edge_index, W, b):
    plan, in_maps = preprocess(x, edge_index, W, b)
    sig = tuple(tuple(s) for s in plan.pass_sizes)
    ent = _CACHE.get("prog")
    if ent is None or ent[0] != sig:
        nc = build_program(plan)
        _CACHE["prog"] = (sig, nc)
    nc = _CACHE["prog"][1]
    out, _ = run(plan, nc, in_maps)
    return out
